# revision 27
# baseline (speedup 1.0000x reference)
"""Distributed multi-head attention kernel for 8 TRN2 NeuronCores.

Problem: x [4, 2048, 1024] -> qkv proj -> 16-head attention (d=64)
         -> out proj + bias -> [4, 2048, 1024].

Sharding (no collectives): core i handles batch b = i//2 and head-half
hh = i%2 (8 of the 16 heads, ALL 2048 query tokens). Each core projects
Q/K/V only for its own 8 heads (columns hh*512..hh*512+512 of each
block of w_qkv), runs attention for those heads over the full sequence,
and applies the out-projection restricted to its heads' rows of w_out.
The two cores of a batch produce additive partial outputs; the host
sums them and adds the bias.

Per-core pipeline (bf16 on the TensorE, fp32 PSUM accum), 8 local heads
= 4 pairs, each pair's two heads stacked on SBUF partitions 0:64 /
64:128 of the Q^T/K^T tiles:

  proj:  full-efficiency K=128 matmuls; V keeps a ones column per head
         so the PV matmul yields softmax denominators for free.
  attn:  per (pair, 512-query chunk, 128-key chunk):
           S^T via TWO row-tiled 64x128 matmuls (head0 on PE rows 0:63,
           head1 on rows 64:127 -- they execute CONCURRENTLY in the
           2x-row-tiled PE array), one exp() on the ScalarE over both
           heads' scores [128, 2, 512] (N=1024 per ACTIVATE), then two
           PV matmuls accumulating U^T[65, 512] per head (row 64 = the
           softmax denominator).
         The ScalarE exp stream is the critical resource (~294us); all
         projection and out-projection matmuls are interleaved into the
         PE's idle time underneath it.
  norm:  1/D via the fast DVE reciprocal, broadcast via a K=1 f32r
         matmul, multiply on the GpSimd engine straight into packed
         [128, 2048] per-pair tiles (so the out-proj contracts K=128).
  out:   pass A (pairs 0+1) runs as filler during pair 2/3 attention
         into resident f32 tiles; pass B (pairs 2+3) adds on top and
         streams to DRAM.
"""

import numpy as np
import ml_dtypes

B = 4
N = 2048
DIM = 1024
HEADS = 16
DH = 64
NCORES = 8
LHEADS = 8    # heads per core
PAIRS = 4     # head pairs per core

_CACHE = {}


def _build_nc():
    from contextlib import ExitStack

    import concourse.bass as bass
    import concourse.mybir as mybir
    import concourse.tile as tile
    from concourse import bacc

    f32 = mybir.dt.float32
    f16 = mybir.dt.float16
    bf16 = mybir.dt.bfloat16
    EXP = mybir.ActivationFunctionType.Exp

    nc = bacc.Bacc("TRN2", target_bir_lowering=False, debug=False,
                   num_devices=NCORES)

    xt_d = nc.dram_tensor("xt", [DIM, N], bf16, kind="ExternalInput")
    # per-core slices of w_qkv: [DIM, 512] each for q, k, v
    wq_d = nc.dram_tensor("wq", [DIM, 512], bf16, kind="ExternalInput")
    wk_d = nc.dram_tensor("wk", [DIM, 512], bf16, kind="ExternalInput")
    wv_d = nc.dram_tensor("wv", [DIM, 512], bf16, kind="ExternalInput")
    wo_d = nc.dram_tensor("wo", [PAIRS, 128, DIM], bf16, kind="ExternalInput")
    out_d = nc.dram_tensor("out", [N, DIM], f16, kind="ExternalOutput")

    with tile.TileContext(nc) as tc, ExitStack() as top:
        const_pool = top.enter_context(tc.tile_pool(name="const", bufs=1))
        mm_psum = top.enter_context(tc.tile_pool(name="mmps", bufs=2, space="PSUM"))
        sp0_psum = top.enter_context(tc.tile_pool(name="sp0", bufs=1, space="PSUM"))
        sp1_psum = top.enter_context(tc.tile_pool(name="sp1", bufs=1, space="PSUM"))
        u_psum = top.enter_context(tc.tile_pool(name="ups", bufs=2, space="PSUM"))
        es0_pool = top.enter_context(tc.tile_pool(name="es0", bufs=2))
        es1_pool = top.enter_context(tc.tile_pool(name="es1", bufs=2))
        nrm_pool = top.enter_context(tc.tile_pool(name="nrm", bufs=4))
        upk_pool = top.enter_context(tc.tile_pool(name="upk", bufs=1))
        vt_pool = top.enter_context(tc.tile_pool(name="vt", bufs=1))
        wo_pool = top.enter_context(tc.tile_pool(name="wo", bufs=1))

        ones_f = const_pool.tile([1, 64], bf16, tag="ones_f", name="ones_f")
        nc.gpsimd.memset(ones_f[:], 1.0)

        # ---------------- DMA: v weights + xt(t0) first ----------------
        xt_pool = tc.alloc_tile_pool(name="xt", bufs=1)
        w_pool = tc.alloc_tile_pool(name="w", bufs=1)
        # weights as single multi-slot tiles: [128 part, fc, cols] -- each
        # loads with ONE descriptor (the ~600ns/DMA cost is fixed overhead)
        WQ = w_pool.tile([128, 8, 512], bf16, tag="wq", name="wq")
        WK = w_pool.tile([128, 8, 512], bf16, tag="wk", name="wk")
        WV = w_pool.tile([128, 8, 512], bf16, tag="wv", name="wv")
        # xt split into the t=0 token chunk (unblocks V/K/Q pair-0 fast)
        # and the rest; separate tiles so dependencies don't conflate them
        XT0 = xt_pool.tile([128, 8, 512], bf16, tag="xt0", name="xt0")
        XT1 = xt_pool.tile([128, 8, 1536], bf16, tag="xt1", name="xt1")

        def xt_ap(fc, lo, width):
            """AP for xt[fc*128:(fc+1)*128, lo:lo+width] (token columns)."""
            if lo + width <= 512:
                return XT0[:, fc, lo:lo + width]
            assert lo >= 512
            return XT1[:, fc, lo - 512:lo - 512 + width]

        # weights on the scalar-engine HWDGE queue (ACT idle in prologue),
        # activations on the sync-engine queue -- two DMA streams in parallel
        xt_r = xt_d.ap().rearrange("(f p) c -> p f c", p=128)
        nc.sync.dma_start(XT0[:], xt_r[:, :, 0:512])
        nc.scalar.dma_start(WV[:], wv_d.ap().rearrange("(f p) c -> p f c", p=128))
        nc.gpsimd.dma_start(WK[:], wk_d.ap().rearrange("(f p) c -> p f c", p=128))
        nc.scalar.dma_start(WQ[:], wq_d.ap().rearrange("(f p) c -> p f c", p=128))
        nc.sync.dma_start(XT1[:], xt_r[:, :, 512:2048])
        WO = [wo_pool.tile([128, DIM], bf16, tag=f"wo{p}", name=f"wo{p}")
              for p in range(PAIRS)]
        for p in range(PAIRS):
            nc.scalar.dma_start(WO[p][:], wo_d.ap()[p])

        # ---------------- projection units ----------------
        qkv_pool = tc.alloc_tile_pool(name="qkv", bufs=1, side="right")
        QT = [qkv_pool.tile([128, N], bf16, tag=f"q{p}", name=f"q{p}")
              for p in range(PAIRS)]
        KT = [qkv_pool.tile([128, N], bf16, tag=f"k{p}", name=f"k{p}")
              for p in range(PAIRS)]
        VT = [vt_pool.tile([128, LHEADS, 65], bf16, tag=f"v{mk}",
                           name=f"v{mk}") for mk in range(16)]

        def v_unit(mk):
            ps = mm_psum.tile([128, 512], f32, tag="mm", name="mm")
            for fc in range(8):
                nc.tensor.matmul(
                    ps[:], xt_ap(fc, mk * 128, 128),
                    WV[:, fc, :], start=(fc == 0), stop=(fc == 7))
            nc.vector.tensor_copy(
                VT[mk][:, :, 0:64],
                ps[:].rearrange("p (h d) -> p h d", d=64))
            nc.gpsimd.memset(VT[mk][:, :, 64:65], 1.0)

        def qk_unit(wb, dest, p, t):
            ps = mm_psum.tile([128, 512], f32, tag="mm", name="mm")
            for fc in range(8):
                nc.tensor.matmul(
                    ps[:], wb[:, fc, p * 128:(p + 1) * 128],
                    xt_ap(fc, t * 512, 512), start=(fc == 0), stop=(fc == 7))
            nc.vector.tensor_copy(dest[p][:, t * 512:(t + 1) * 512], ps[:])

        # ---------------- out-projection units ----------------
        state = {}

        def passA(tc_i, of):
            ps = mm_psum.tile([128, 512], f32, tag="mm", name="mm")
            for p in range(2):
                nc.tensor.matmul(
                    ps[:], state["UPK"][p][:, tc_i * 128:(tc_i + 1) * 128],
                    WO[p][:, of * 512:(of + 1) * 512],
                    start=(p == 0), stop=(p == 1))
            nc.vector.tensor_copy(
                state["FIN"][tc_i][:, of * 512:(of + 1) * 512], ps[:])

        def passB(tc_i, of, can_use_scalar_q=False):
            ps = mm_psum.tile([128, 512], f32, tag="mm", name="mm")
            for p in range(2, 4):
                nc.tensor.matmul(
                    ps[:], state["UPK"][p][:, tc_i * 128:(tc_i + 1) * 128],
                    WO[p][:, of * 512:(of + 1) * 512],
                    start=(p == 2), stop=(p == 3))
            fin = state["FIN"][tc_i]
            with nc.allow_low_precision(reason="f16 partial output"):
                nc.vector.tensor_add(
                    fin[:, of * 512:(of + 1) * 512],
                    fin[:, of * 512:(of + 1) * 512], ps[:])
            if of == 1:
                # while exp is still streaming, the scalar HWDGE queue would
                # insert DMA bubbles into the ACTIVATE stream -- sync only
                eng = nc.scalar if (can_use_scalar_q and tc_i % 2) else nc.sync
                eng.dma_start(
                    out_d.ap()[tc_i * 128:(tc_i + 1) * 128, :], fin[:])

        # ---------------- attention ----------------
        UPK = [upk_pool.tile([128, N], bf16, tag=f"upk{p}", name=f"upk{p}")
               for p in range(PAIRS)]
        state["UPK"] = UPK

        def norm_drain(p, qc, h, u):
            """DVE-only drain of the PSUM accumulator (frees the banks for
            the next block). Returns args for norm_finish."""
            ur_sb = nrm_pool.tile([64, 512], bf16, tag="ur", name="ur_sb")
            nc.vector.tensor_copy(ur_sb[:], u[0:64, :])
            # custom-DVE ops read from partition 0 of the AP's buffer, so
            # stage the denominator row into its own partition-0 tile first
            d_sb = nrm_pool.tile([1, 512], f32, tag="dsb", name="d_sb")
            nc.vector.tensor_copy(d_sb[:], u[64:65, :])
            return (p, qc, h, ur_sb, d_sb)

        def norm_finish(p, qc, h, ur_sb, d_sb):
            """Reciprocal + broadcast + multiply. Deferred into the next
            block so the bc matmul never makes the PE wait on the DVE."""
            rec = nrm_pool.tile([1, 512], f32, tag="rec", name="rec")
            nc.vector.reciprocal_approx_fast(rec[:], d_sb[:])
            rec_bf = nrm_pool.tile([1, 512], bf16, tag="recb", name="rec_bf")
            nc.vector.tensor_copy(rec_bf[:], rec[:])
            bc = mm_psum.tile([64, 512], f32, tag="mm", name="bc")
            nc.tensor.matmul(bc[:], ones_f[:], rec_bf[:],
                             start=True, stop=True)
            bc_sb = nrm_pool.tile([64, 512], f32, tag="bc", name="bc_sb")
            nc.vector.tensor_copy(bc_sb[:], bc[:])
            nc.gpsimd.tensor_mul(
                UPK[p][64 * h:64 * h + 64, qc * 512:(qc + 1) * 512],
                ur_sb[:], bc_sb[:])

        pending = []   # norm_finish args deferred from the previous block
        u_tiles = {}   # (p, qc) -> (u0, u1) PSUM accumulators

        def emit_pv(p, qc, kk, es0, es1, units):
            """PV for double-chunk kk of block (p, qc) -- emitted one
            iteration LATE so the PE never FIFO-blocks on an exp. The
            `units` (proj / out-proj fillers) interleave between the two
            heads' PV pairs, landing in the PE's exp-shadow slack."""
            if kk == 0:
                u_tiles[(p, qc)] = (
                    u_psum.tile([65, 512], f32, tag="u", name="u0"),
                    u_psum.tile([65, 512], f32, tag="u", name="u1"))
            u0, u1 = u_tiles[(p, qc)]
            k0, k1 = 2 * kk, 2 * kk + 1
            nc.tensor.matmul(u0[:], VT[k0][:, 2 * p, :], es0[:, 0, :],
                             start=(kk == 0), stop=False)
            nc.tensor.matmul(u0[:], VT[k1][:, 2 * p, :], es0[:, 1, :],
                             start=False, stop=(kk == 7))
            for f in units[:1]:
                f()
            nc.tensor.matmul(u1[:], VT[k0][:, 2 * p + 1, :], es1[:, 0, :],
                             start=(kk == 0), stop=False)
            nc.tensor.matmul(u1[:], VT[k1][:, 2 * p + 1, :], es1[:, 1, :],
                             start=False, stop=(kk == 7))
            for f in units[1:]:
                f()
            if kk == 7:
                u0, u1 = u_tiles.pop((p, qc))
                pending.append(norm_drain(p, qc, 0, u0))
                pending.append(norm_drain(p, qc, 1, u1))

        # ---------------- emission ----------------
        # minimal prologue: V chunks 0-1, K pair0 t0, Q pair0 t0 -- just
        # enough for block (0,0) to start; everything else is a filler
        v_unit(0)
        v_unit(1)
        qk_unit(WK, KT, 0, 0)
        qk_unit(WQ, QT, 0, 0)

        V = lambda mk: (lambda: v_unit(mk))
        K = lambda p, t: (lambda: qk_unit(WK, KT, p, t))
        Q = lambda p, t: (lambda: qk_unit(WQ, QT, p, t))

        # block (0,0) placement, keyed by iteration kk: V(2kk)/V(2kk+1)
        # pop at iteration kk (consumed by emit_pv(kk) one iteration
        # later); K-pair0 t-chunk j pops one iteration before S needs it
        placed = {(0, 0, 1): [K(0, 1), V(2), V(3)],
                  (0, 0, 2): [V(4), V(5)],
                  (0, 0, 3): [V(6), V(7), K(0, 2)],
                  (0, 0, 4): [V(8), V(9)],
                  (0, 0, 5): [V(10), V(11), K(0, 3)],
                  (0, 0, 6): [V(12), V(13)],
                  (0, 0, 7): [V(14), V(15), Q(0, 1)]}

        # remaining proj + out-proj units, spread per block
        sched = {}
        rest = ([Q(0, 2), Q(0, 3)]
                + [K(1, t) for t in range(4)] + [Q(1, t) for t in range(4)]
                + [K(2, t) for t in range(4)] + [Q(2, t) for t in range(4)]
                + [K(3, t) for t in range(4)] + [Q(3, t) for t in range(4)])
        blocks = [(p, qc) for p in range(3) for qc in range(4)][1:]
        per = (len(rest) + len(blocks) - 1) // len(blocks)
        for i, blk in enumerate(blocks):
            sched[blk] = rest[i * per:(i + 1) * per]

        def setup_p3():
            # proj inputs are dead; make room and set up out-proj pass A
            w_pool.release()
            xt_pool.release()
            state["st_pool"] = tc.alloc_tile_pool(name="st", bufs=2)
            state["FIN"] = [
                state["st_pool"].tile([128, DIM], f16, tag=f"fin{i}",
                                      name=f"fin{i}", bufs=1)
                for i in range(16)]

        pA = [lambda i=i, of=of: passA(i, of)
              for i in range(16) for of in range(2)]
        pB = [lambda i=i, of=of: passB(i, of)
              for i in range(12) for of in range(2)]
        sched[(3, 0)] = pA[:16]
        sched[(3, 1)] = pA[16:32]
        sched[(3, 2)] = pB[:8]     # tc 0-3  (needs pair-3 qc0 norm)
        sched[(3, 3)] = pB[8:24]   # tc 4-11 (needs pair-3 qc1/qc2 norms)

        prev = None   # (p, qc, kk, es0, es1) whose PV is outstanding
        for p in range(PAIRS):
            for qc in range(4):
                if (p, qc) == (3, 0):
                    setup_p3()
                blkq = list(sched.get((p, qc), []))
                nblk = len(blkq)
                popped = 0
                for kk in range(8):
                    sp0 = sp0_psum.tile([128, 2, 512], f32, tag="sp0",
                                        name="sp0")
                    sp1 = sp1_psum.tile([128, 2, 512], f32, tag="sp1",
                                        name="sp1")
                    # interleave heads so each adjacent S pair occupies
                    # disjoint PE row halves and runs CONCURRENTLY
                    for j, k in enumerate((2 * kk, 2 * kk + 1)):
                        nc.tensor.matmul(
                            sp0[:, j, :], KT[p][0:64, k * 128:(k + 1) * 128],
                            QT[p][0:64, qc * 512:(qc + 1) * 512],
                            start=True, stop=True)
                        nc.tensor.matmul(
                            sp1[:, j, :], KT[p][64:128, k * 128:(k + 1) * 128],
                            QT[p][64:128, qc * 512:(qc + 1) * 512],
                            start=True, stop=True)
                    es0 = es0_pool.tile([128, 2, 512], bf16, tag="es0",
                                        name="es0")
                    nc.scalar.activation(es0[:], sp0[:], EXP, scale=0.125)
                    es1 = es1_pool.tile([128, 2, 512], bf16, tag="es1",
                                        name="es1")
                    nc.scalar.activation(es1[:], sp1[:], EXP, scale=0.125)

                    units = list(placed.get((p, qc, kk), []))
                    take = 0
                    while blkq and (popped + take) * 14 < (2 * kk) * nblk:
                        units.append(blkq.pop(0))
                        take += 1
                    popped += take
                    if prev is not None:
                        emit_pv(*prev, units)
                    elif units:
                        for f in units:
                            f()
                    prev = (p, qc, kk, es0, es1)
                    if kk == 1:
                        while pending:
                            norm_finish(*pending.pop(0))
                for f in blkq:
                    f()

        emit_pv(*prev, [])
        while pending:
            norm_finish(*pending.pop(0))
        for i in range(12, 16):
            for of in range(2):
                passB(i, of, can_use_scalar_q=True)

        state["st_pool"].release()
        qkv_pool.release()

    nc.compile()
    return nc


def _get_nc():
    if "nc" not in _CACHE:
        _CACHE["nc"] = _build_nc()
    return _CACHE["nc"]


def _make_in_maps(x, w_qkv, w_out, b_out):
    bf = ml_dtypes.bfloat16
    w_qkv = np.asarray(w_qkv, np.float32)
    w_out = np.asarray(w_out, np.float32)
    halves = []
    for hh in range(2):
        c0 = hh * 512
        wq = np.ascontiguousarray(w_qkv[:, c0:c0 + 512]).astype(bf)
        wk = np.ascontiguousarray(w_qkv[:, DIM + c0:DIM + c0 + 512]).astype(bf)
        wv = np.ascontiguousarray(
            w_qkv[:, 2 * DIM + c0:2 * DIM + c0 + 512]).astype(bf)
        wo = np.ascontiguousarray(
            w_out[c0:c0 + 512, :].reshape(PAIRS, 128, DIM)).astype(bf)
        halves.append((wq, wk, wv, wo))
    in_maps = []
    for i in range(NCORES):
        b, hh = i // 2, i % 2
        xt = np.ascontiguousarray(np.asarray(x[b], np.float32).T.astype(bf))
        wq, wk, wv, wo = halves[hh]
        in_maps.append({"xt": xt, "wq": wq, "wk": wk, "wv": wv, "wo": wo})
    return in_maps


def _assemble(results, b_out):
    out = np.empty((B, N, DIM), np.float32)
    bias = np.asarray(b_out, np.float32)
    for b in range(B):
        out[b] = (np.asarray(results[2 * b]["out"], np.float32)
                  + np.asarray(results[2 * b + 1]["out"], np.float32) + bias)
    return out


def run(x, w_qkv, w_out, b_out, trace=False):
    """Run the kernel; returns (output, BassKernelResults)."""
    from concourse.bass_utils import run_bass_kernel_spmd
    nc = _get_nc()
    in_maps = _make_in_maps(x, w_qkv, w_out, b_out)
    res = run_bass_kernel_spmd(nc, in_maps, core_ids=list(range(NCORES)),
                               trace=trace)
    return _assemble(res.results, b_out), res


def kernel(x, w_qkv, w_out, b_out):
    out, _ = run(x, w_qkv, w_out, b_out, trace=False)
    return out


# revision 31
# speedup vs baseline: 1.2892x; 1.2892x over previous
"""Distributed multi-head attention kernel for 8 TRN2 NeuronCores.

Problem: x [4, 2048, 1024] -> qkv proj -> 16-head attention (d=64)
         -> out proj + bias -> [4, 2048, 1024].

Sharding (no collectives): core i handles batch b = i//2 and head-half
hh = i%2 (8 of the 16 heads, ALL 2048 query tokens). Each core projects
Q/K/V only for its own 8 heads (columns hh*512..hh*512+512 of each
block of w_qkv), runs attention for those heads over the full sequence,
and applies the out-projection restricted to its heads' rows of w_out.
The two cores of a batch produce additive partial outputs; the host
sums them and adds the bias.

Per-core pipeline (bf16 on the TensorE, fp32 PSUM accum), 8 local heads
= 4 pairs, each pair's two heads stacked on SBUF partitions 0:64 /
64:128 of the Q^T/K^T tiles:

  proj:  full-efficiency K=128 matmuls; V keeps a ones column per head
         so the PV matmul yields softmax denominators for free.
  attn:  per (pair, 512-query chunk, 128-key chunk):
           S^T via TWO row-tiled 64x128 matmuls (head0 on PE rows 0:63,
           head1 on rows 64:127 -- they execute CONCURRENTLY in the
           2x-row-tiled PE array), one exp() on the ScalarE over both
           heads' scores [128, 2, 512] (N=1024 per ACTIVATE), then two
           PV matmuls accumulating U^T[65, 512] per head (row 64 = the
           softmax denominator).
         The ScalarE exp stream is the critical resource (~294us); all
         projection and out-projection matmuls are interleaved into the
         PE's idle time underneath it.
  norm:  1/D via the fast DVE reciprocal, broadcast via a K=1 f32r
         matmul, multiply on the GpSimd engine straight into packed
         [128, 2048] per-pair tiles (so the out-proj contracts K=128).
  out:   pass A (pairs 0+1) runs as filler during pair 2/3 attention
         into resident f32 tiles; pass B (pairs 2+3) adds on top and
         streams to DRAM.
"""

import numpy as np
import ml_dtypes

B = 4
N = 2048
DIM = 1024
HEADS = 16
DH = 64
NCORES = 8
LHEADS = 8    # heads per core
PAIRS = 4     # head pairs per core

_CACHE = {}


def _build_nc():
    from contextlib import ExitStack

    import concourse.bass as bass
    import concourse.mybir as mybir
    import concourse.tile as tile
    from concourse import bacc

    f32 = mybir.dt.float32
    f16 = mybir.dt.float16
    bf16 = mybir.dt.bfloat16
    EXP = mybir.ActivationFunctionType.Exp

    nc = bacc.Bacc("TRN2", target_bir_lowering=False, debug=False,
                   num_devices=NCORES)

    xt_d = nc.dram_tensor("xt", [DIM, N], bf16, kind="ExternalInput")
    # per-core slices of w_qkv: [DIM, 512] each for q, k, v
    wq_d = nc.dram_tensor("wq", [DIM, 512], bf16, kind="ExternalInput")
    wk_d = nc.dram_tensor("wk", [DIM, 512], bf16, kind="ExternalInput")
    wv_d = nc.dram_tensor("wv", [DIM, 512], bf16, kind="ExternalInput")
    wo_d = nc.dram_tensor("wo", [PAIRS, 128, DIM], bf16, kind="ExternalInput")
    out_d = nc.dram_tensor("out", [N, DIM], f16, kind="ExternalOutput")

    with tile.TileContext(nc) as tc, ExitStack() as top:
        const_pool = top.enter_context(tc.tile_pool(name="const", bufs=1))
        mm_psum = top.enter_context(tc.tile_pool(name="mmps", bufs=2, space="PSUM"))
        sp_psum = top.enter_context(tc.tile_pool(name="spps", bufs=2, space="PSUM"))
        u_psum = top.enter_context(tc.tile_pool(name="ups", bufs=2, space="PSUM"))
        es_pool = top.enter_context(tc.tile_pool(name="es", bufs=4))
        nrm_pool = top.enter_context(tc.tile_pool(name="nrm", bufs=4))
        upk_pool = top.enter_context(tc.tile_pool(name="upk", bufs=1))
        vt_pool = top.enter_context(tc.tile_pool(name="vt", bufs=1))
        wo_pool = top.enter_context(tc.tile_pool(name="wo", bufs=1))

        ones_f = const_pool.tile([1, 64], bf16, tag="ones_f", name="ones_f")
        nc.gpsimd.memset(ones_f[:], 1.0)

        # ---------------- DMA: v weights + xt(t0) first ----------------
        xt_pool = tc.alloc_tile_pool(name="xt", bufs=1)
        w_pool = tc.alloc_tile_pool(name="w", bufs=1)
        # weights as single multi-slot tiles: [128 part, fc, cols] -- each
        # loads with ONE descriptor (the ~600ns/DMA cost is fixed overhead)
        WQ = w_pool.tile([128, 8, 512], bf16, tag="wq", name="wq")
        WK = w_pool.tile([128, 8, 512], bf16, tag="wk", name="wk")
        WV = w_pool.tile([128, 8, 512], bf16, tag="wv", name="wv")
        # xt split into the t=0 token chunk (unblocks V/K/Q pair-0 fast)
        # and the rest; separate tiles so dependencies don't conflate them
        XT0 = xt_pool.tile([128, 8, 512], bf16, tag="xt0", name="xt0")
        XT1 = xt_pool.tile([128, 8, 1536], bf16, tag="xt1", name="xt1")

        def xt_ap(fc, lo, width):
            """AP for xt[fc*128:(fc+1)*128, lo:lo+width] (token columns)."""
            if lo + width <= 512:
                return XT0[:, fc, lo:lo + width]
            assert lo >= 512
            return XT1[:, fc, lo - 512:lo - 512 + width]

        # weights on the scalar-engine HWDGE queue (ACT idle in prologue),
        # activations on the sync-engine queue -- two DMA streams in parallel
        xt_r = xt_d.ap().rearrange("(f p) c -> p f c", p=128)
        nc.sync.dma_start(XT0[:], xt_r[:, :, 0:512])
        nc.scalar.dma_start(WV[:], wv_d.ap().rearrange("(f p) c -> p f c", p=128))
        nc.gpsimd.dma_start(WK[:], wk_d.ap().rearrange("(f p) c -> p f c", p=128))
        nc.scalar.dma_start(WQ[:], wq_d.ap().rearrange("(f p) c -> p f c", p=128))
        nc.sync.dma_start(XT1[:], xt_r[:, :, 512:2048])
        WO = [wo_pool.tile([128, DIM], bf16, tag=f"wo{p}", name=f"wo{p}")
              for p in range(PAIRS)]
        for p in range(PAIRS):
            nc.scalar.dma_start(WO[p][:], wo_d.ap()[p])

        # ---------------- projection units ----------------
        qkv_pool = tc.alloc_tile_pool(name="qkv", bufs=1, side="right")
        QT = [qkv_pool.tile([128, N], bf16, tag=f"q{p}", name=f"q{p}")
              for p in range(PAIRS)]
        KT = [qkv_pool.tile([128, N], bf16, tag=f"k{p}", name=f"k{p}")
              for p in range(PAIRS)]
        VT = [vt_pool.tile([128, LHEADS, 65], bf16, tag=f"v{mk}",
                           name=f"v{mk}") for mk in range(16)]

        def v_unit(mk):
            ps = mm_psum.tile([128, 512], f32, tag="mm", name="mm")
            for fc in range(8):
                nc.tensor.matmul(
                    ps[:], xt_ap(fc, mk * 128, 128),
                    WV[:, fc, :], start=(fc == 0), stop=(fc == 7))
            nc.vector.tensor_copy(
                VT[mk][:, :, 0:64],
                ps[:].rearrange("p (h d) -> p h d", d=64))
            nc.gpsimd.memset(VT[mk][:, :, 64:65], 1.0)

        def qk_unit(wb, dest, p, t):
            ps = mm_psum.tile([128, 512], f32, tag="mm", name="mm")
            for fc in range(8):
                nc.tensor.matmul(
                    ps[:], wb[:, fc, p * 128:(p + 1) * 128],
                    xt_ap(fc, t * 512, 512), start=(fc == 0), stop=(fc == 7))
            nc.vector.tensor_copy(dest[p][:, t * 512:(t + 1) * 512], ps[:])

        # ---------------- out-projection units ----------------
        state = {}

        def passA(tc_i, of):
            ps = mm_psum.tile([128, 512], f32, tag="mm", name="mm")
            for p in range(2):
                nc.tensor.matmul(
                    ps[:], state["UPK"][p][:, tc_i * 128:(tc_i + 1) * 128],
                    WO[p][:, of * 512:(of + 1) * 512],
                    start=(p == 0), stop=(p == 1))
            nc.vector.tensor_copy(
                state["FIN"][tc_i][:, of * 512:(of + 1) * 512], ps[:])

        def passB(tc_i, of, can_use_scalar_q=False):
            ps = mm_psum.tile([128, 512], f32, tag="mm", name="mm")
            for p in range(2, 4):
                nc.tensor.matmul(
                    ps[:], state["UPK"][p][:, tc_i * 128:(tc_i + 1) * 128],
                    WO[p][:, of * 512:(of + 1) * 512],
                    start=(p == 2), stop=(p == 3))
            fin = state["FIN"][tc_i]
            with nc.allow_low_precision(reason="f16 partial output"):
                nc.vector.tensor_add(
                    fin[:, of * 512:(of + 1) * 512],
                    fin[:, of * 512:(of + 1) * 512], ps[:])
            if of == 1:
                # while exp is still streaming, the scalar HWDGE queue would
                # insert DMA bubbles into the ACTIVATE stream -- sync only
                eng = nc.scalar if (can_use_scalar_q and tc_i % 2) else nc.sync
                eng.dma_start(
                    out_d.ap()[tc_i * 128:(tc_i + 1) * 128, :], fin[:])

        # ---------------- attention ----------------
        UPK = [upk_pool.tile([128, N], bf16, tag=f"upk{p}", name=f"upk{p}")
               for p in range(PAIRS)]
        state["UPK"] = UPK

        def norm_drain(p, qc, h, u):
            """DVE-only drain of the PSUM accumulator (frees the banks for
            the next block). Returns args for norm_finish."""
            ur_sb = nrm_pool.tile([64, 512], bf16, tag="ur", name="ur_sb")
            nc.vector.tensor_copy(ur_sb[:], u[0:64, :])
            # custom-DVE ops read from partition 0 of the AP's buffer, so
            # stage the denominator row into its own partition-0 tile first
            d_sb = nrm_pool.tile([1, 512], f32, tag="dsb", name="d_sb")
            nc.vector.tensor_copy(d_sb[:], u[64:65, :])
            return (p, qc, h, ur_sb, d_sb)

        def norm_finish(p, qc, h, ur_sb, d_sb):
            """Reciprocal + broadcast + multiply. Deferred into the next
            block so the bc matmul never makes the PE wait on the DVE."""
            rec = nrm_pool.tile([1, 512], f32, tag="rec", name="rec")
            nc.vector.reciprocal_approx_fast(rec[:], d_sb[:])
            rec_bf = nrm_pool.tile([1, 512], bf16, tag="recb", name="rec_bf")
            nc.vector.tensor_copy(rec_bf[:], rec[:])
            bc = mm_psum.tile([64, 512], f32, tag="mm", name="bc")
            nc.tensor.matmul(bc[:], ones_f[:], rec_bf[:],
                             start=True, stop=True)
            bc_sb = nrm_pool.tile([64, 512], f32, tag="bc", name="bc_sb")
            nc.vector.tensor_copy(bc_sb[:], bc[:])
            nc.gpsimd.tensor_mul(
                UPK[p][64 * h:64 * h + 64, qc * 512:(qc + 1) * 512],
                ur_sb[:], bc_sb[:])

        pending = []   # norm_finish args deferred from the previous block
        u_tiles = {}   # (p, qc) -> (u0, u1) PSUM accumulators

        def emit_pv(p, qc, k, es, units):
            """PV for key-chunk k of block (p, qc) -- emitted one chunk
            LATE so the PE never FIFO-blocks on the exp it consumes. The
            `units` (proj / out-proj fillers) follow, landing in the PE's
            exp-shadow slack."""
            if k == 0:
                u_tiles[(p, qc)] = (
                    u_psum.tile([65, 512], f32, tag="u", name="u0"),
                    u_psum.tile([65, 512], f32, tag="u", name="u1"))
            u0, u1 = u_tiles[(p, qc)]
            nc.tensor.matmul(u0[:], VT[k][:, 2 * p, :], es[:, 0, :],
                             start=(k == 0), stop=(k == 15))
            nc.tensor.matmul(u1[:], VT[k][:, 2 * p + 1, :], es[:, 1, :],
                             start=(k == 0), stop=(k == 15))
            for f in units:
                f()
            if k == 15:
                u0, u1 = u_tiles.pop((p, qc))
                pending.append(norm_drain(p, qc, 0, u0))
                pending.append(norm_drain(p, qc, 1, u1))

        # ---------------- emission ----------------
        # minimal prologue: V chunks 0-1, K pair0 t0, Q pair0 t0 -- just
        # enough for block (0,0) to start; everything else is a filler
        v_unit(0)
        v_unit(1)
        qk_unit(WK, KT, 0, 0)
        qk_unit(WQ, QT, 0, 0)

        V = lambda mk: (lambda: v_unit(mk))
        K = lambda p, t: (lambda: qk_unit(WK, KT, p, t))
        Q = lambda p, t: (lambda: qk_unit(WQ, QT, p, t))

        # block (0,0) placement, keyed by chunk k: V(j) pops at k=j-1
        # (consumed by emit_pv(j) one chunk later); K-pair0 t-chunk j
        # pops well before S(4j) needs it
        placed = {(0, 0, 0): [K(0, 1)],
                  (0, 0, 1): [V(2), V(3)],
                  (0, 0, 2): [V(4)], (0, 0, 3): [V(5)],
                  (0, 0, 4): [V(6)], (0, 0, 5): [V(7), K(0, 2)],
                  (0, 0, 6): [V(8)], (0, 0, 7): [V(9)],
                  (0, 0, 8): [V(10)], (0, 0, 9): [V(11), K(0, 3)],
                  (0, 0, 10): [V(12)], (0, 0, 11): [V(13)],
                  (0, 0, 12): [V(14)], (0, 0, 13): [V(15)],
                  (0, 0, 14): [Q(0, 1)]}

        # remaining proj + out-proj units, spread per block
        sched = {}
        rest = ([Q(0, 2), Q(0, 3)]
                + [K(1, t) for t in range(4)] + [Q(1, t) for t in range(4)]
                + [K(2, t) for t in range(4)] + [Q(2, t) for t in range(4)]
                + [K(3, t) for t in range(4)] + [Q(3, t) for t in range(4)])
        blocks = [(p, qc) for p in range(3) for qc in range(4)][1:]
        per = (len(rest) + len(blocks) - 1) // len(blocks)
        for i, blk in enumerate(blocks):
            sched[blk] = rest[i * per:(i + 1) * per]

        def setup_p3():
            # proj inputs are dead; make room and set up out-proj pass A
            w_pool.release()
            xt_pool.release()
            state["st_pool"] = tc.alloc_tile_pool(name="st", bufs=2)
            state["FIN"] = [
                state["st_pool"].tile([128, DIM], f16, tag=f"fin{i}",
                                      name=f"fin{i}", bufs=1)
                for i in range(16)]

        pA = [lambda i=i, of=of: passA(i, of)
              for i in range(16) for of in range(2)]
        pB = [lambda i=i, of=of: passB(i, of)
              for i in range(12) for of in range(2)]
        sched[(3, 0)] = pA[:16]
        sched[(3, 1)] = pA[16:32]
        sched[(3, 2)] = pB[:8]     # tc 0-3  (needs pair-3 qc0 norm)
        sched[(3, 3)] = pB[8:24]   # tc 4-11 (needs pair-3 qc1/qc2 norms)

        prev = None   # (p, qc, k, es) whose PV is outstanding
        for p in range(PAIRS):
            for qc in range(4):
                if (p, qc) == (3, 0):
                    setup_p3()
                blkq = list(sched.get((p, qc), []))
                nblk = len(blkq)
                popped = 0
                for k in range(16):
                    # both heads' S into ONE tile: the adjacent matmuls
                    # occupy disjoint PE row halves, share their wait
                    # set, and dual-issue (2x row tiling)
                    sp = sp_psum.tile([128, 2, 512], f32, tag="sp",
                                      name="sp")
                    nc.tensor.matmul(
                        sp[:, 0, :], KT[p][0:64, k * 128:(k + 1) * 128],
                        QT[p][0:64, qc * 512:(qc + 1) * 512],
                        start=True, stop=True)
                    nc.tensor.matmul(
                        sp[:, 1, :], KT[p][64:128, k * 128:(k + 1) * 128],
                        QT[p][64:128, qc * 512:(qc + 1) * 512],
                        start=True, stop=True)
                    es = es_pool.tile([128, 2, 512], bf16, tag="es",
                                      name="es")
                    nc.scalar.activation(es[:], sp[:], EXP, scale=0.125)

                    units = list(placed.get((p, qc, k), []))
                    take = 0
                    while blkq and (popped + take) * 14 < k * nblk:
                        units.append(blkq.pop(0))
                        take += 1
                    popped += take
                    if prev is not None:
                        emit_pv(*prev, units)
                    elif units:
                        for f in units:
                            f()
                    prev = (p, qc, k, es)
                    if k == 2:
                        while pending:
                            norm_finish(*pending.pop(0))
                for f in blkq:
                    f()

        emit_pv(*prev, [])
        while pending:
            norm_finish(*pending.pop(0))
        for i in range(12, 16):
            for of in range(2):
                passB(i, of, can_use_scalar_q=True)

        state["st_pool"].release()
        qkv_pool.release()

    nc.compile()
    return nc


def _get_nc():
    if "nc" not in _CACHE:
        _CACHE["nc"] = _build_nc()
    return _CACHE["nc"]


def _make_in_maps(x, w_qkv, w_out, b_out):
    bf = ml_dtypes.bfloat16
    w_qkv = np.asarray(w_qkv, np.float32)
    w_out = np.asarray(w_out, np.float32)
    halves = []
    for hh in range(2):
        c0 = hh * 512
        wq = np.ascontiguousarray(w_qkv[:, c0:c0 + 512]).astype(bf)
        wk = np.ascontiguousarray(w_qkv[:, DIM + c0:DIM + c0 + 512]).astype(bf)
        wv = np.ascontiguousarray(
            w_qkv[:, 2 * DIM + c0:2 * DIM + c0 + 512]).astype(bf)
        wo = np.ascontiguousarray(
            w_out[c0:c0 + 512, :].reshape(PAIRS, 128, DIM)).astype(bf)
        halves.append((wq, wk, wv, wo))
    in_maps = []
    for i in range(NCORES):
        b, hh = i // 2, i % 2
        xt = np.ascontiguousarray(np.asarray(x[b], np.float32).T.astype(bf))
        wq, wk, wv, wo = halves[hh]
        in_maps.append({"xt": xt, "wq": wq, "wk": wk, "wv": wv, "wo": wo})
    return in_maps


def _assemble(results, b_out):
    out = np.empty((B, N, DIM), np.float32)
    bias = np.asarray(b_out, np.float32)
    for b in range(B):
        out[b] = (np.asarray(results[2 * b]["out"], np.float32)
                  + np.asarray(results[2 * b + 1]["out"], np.float32) + bias)
    return out


def run(x, w_qkv, w_out, b_out, trace=False):
    """Run the kernel; returns (output, BassKernelResults)."""
    from concourse.bass_utils import run_bass_kernel_spmd
    nc = _get_nc()
    in_maps = _make_in_maps(x, w_qkv, w_out, b_out)
    res = run_bass_kernel_spmd(nc, in_maps, core_ids=list(range(NCORES)),
                               trace=trace)
    return _assemble(res.results, b_out), res


def kernel(x, w_qkv, w_out, b_out):
    out, _ = run(x, w_qkv, w_out, b_out, trace=False)
    return out
